# revision 5
# baseline (speedup 1.0000x reference)
"""Trainium2 Bass kernel for multi-scale average-pool window scoring + NMS proposals.

Problem: x (256,1,112,112) f32 -> 13-ratio sliding-window average scores
(256, 96981), then greedy NMS per 3 ratio-groups picking (2,3,2) boxes.

Device strategy (pure data parallelism, 32 images per core x 8 cores):
  - DVE tensor_tensor_scan: per-row prefix sums  Y[h, j'] = sum_{w<j'} X[h, w]
  - TensorE: scores_r = (s_r*A_rh)^T @ Y[:, j+rw]  -  (s_r*A_rh)^T @ Y[:, j]
    via two PSUM-accumulating matmuls with banded constant stationaries
    (window-sum over rows h as a matmul against the column prefix sums).
  - ScalarE: PSUM -> SBUF evacuation.
  - DMA out the (32, 96981) score shard.
NMS (256x7 int32 indices, 0.006% of output bytes) is computed on host from
the full returned scores with float32 semantics matching the reference.
"""

import os
import numpy as np

# ---------------- problem geometry (hardcoded, matches the nn.Module) -------
FEAT = 112
PSTRIDE = 4
RATIOS = [(16, 16), (12, 20), (20, 12),
          (24, 24), (20, 28), (28, 20),
          (32, 32), (24, 40), (40, 24), (28, 40), (40, 28), (28, 36), (36, 28)]
GROUPS = [(0, 3), (3, 6), (6, 13)]
N_LIST = [2, 3, 2]
IOU_THRESHS = [0.25, 0.25, 0.25]
BATCH = 256
WINDOW_NUMS = [(FEAT - rh + 1) * (FEAT - rw + 1) for rh, rw in RATIOS]
TOTAL = sum(WINDOW_NUMS)  # 96981
PROPOSALN = sum(N_LIST)   # 7

N_CORES = 8
B_PER = BATCH // N_CORES  # 32

NR = [FEAT - rh + 1 for rh, _ in RATIOS]
NC = [FEAT - rw + 1 for _, rw in RATIOS]
NR_OFF = np.cumsum([0] + NR).tolist()   # column offsets into packed A
NRSUM = NR_OFF[-1]                      # 1144
FLAT_OFF = np.cumsum([0] + WINDOW_NUMS).tolist()

_COMPILED = {}


def _build_consts():
    """Packed banded stationary matrices, scaled by +/- 1/(rh*rw)."""
    apos = np.zeros((FEAT, NRSUM), np.float32)
    for r, (rh, rw) in enumerate(RATIOS):
        s = 1.0 / float(rh * rw)
        for i in range(NR[r]):
            apos[i:i + rh, NR_OFF[r] + i] = s
    return apos, -apos


def _build_bass():
    import concourse.bass as bass
    import concourse.bacc as bacc
    import concourse.tile as tile
    import concourse.mybir as mybir

    f32 = mybir.dt.float32
    nc = bacc.Bacc()
    x_ext = nc.declare_dram_parameter("x", [B_PER, FEAT, FEAT], f32, isOutput=False)
    apos_ext = nc.declare_dram_parameter("apos", [FEAT, NRSUM], f32, isOutput=False)
    aneg_ext = nc.declare_dram_parameter("aneg", [FEAT, NRSUM], f32, isOutput=False)
    out_ext = nc.declare_dram_parameter("out", [B_PER, TOTAL], f32, isOutput=True)

    with tile.TileContext(nc) as tc:
        with tc.tile_pool(name="persist", bufs=1) as persist, \
             tc.tile_pool(name="scores", bufs=3) as scores_pool, \
             tc.tile_pool(name="psum", bufs=2, space="PSUM") as psum_pool:

            # constants; staged through a DVE copy so matmuls' lhsT + rhs
            # dependencies collapse onto one semaphore (S3_LW allows a
            # single external sync wait on LDWEIGHTS-carrying matmuls)
            apos_in = persist.tile([FEAT, NRSUM], f32, tag="apos_in")
            aneg_in = persist.tile([FEAT, NRSUM], f32, tag="aneg_in")
            apos = persist.tile([FEAT, NRSUM], f32, tag="apos")
            aneg = persist.tile([FEAT, NRSUM], f32, tag="aneg")
            nc.sync.dma_start(out=apos_in[:], in_=apos_ext[:])
            nc.sync.dma_start(out=aneg_in[:], in_=aneg_ext[:])
            nc.vector.tensor_copy(apos[:], apos_in[:])
            nc.vector.tensor_copy(aneg[:], aneg_in[:])

            # input images: [h=112 partitions, b=32, w=112]
            X = persist.tile([FEAT, B_PER, FEAT], f32, tag="X")
            nc.sync.dma_start(out=X[:], in_=x_ext.rearrange("b h w -> h b w"))

            # row prefix sums with leading zero column: Y[h, b, j'],
            # Y[:, :, 0] = 0, Y[:, :, 1+w] = cumsum_w
            Y = persist.tile([FEAT, B_PER, FEAT + 1], f32, tag="Y")
            nc.vector.memset(Y[:, :, 0], 0.0)
            for b in range(B_PER):
                nc.vector.tensor_tensor_scan(
                    Y[:, b, 1:FEAT + 1], X[:, b, :], X[:, b, :],
                    initial=0.0,
                    op0=mybir.AluOpType.add, op1=mybir.AluOpType.bypass)

            HALF = 16   # images per PSUM round
            GRP = 4     # images per matmul (N = GRP*nc <= 512)
            for r, (rh, rw) in enumerate(RATIOS):
                nr, ncw = NR[r], NC[r]
                lpos = apos[:, NR_OFF[r]:NR_OFF[r] + nr]
                lneg = aneg[:, NR_OFF[r]:NR_OFF[r] + nr]
                for h in range(B_PER // HALF):
                    b0 = h * HALF
                    ps = psum_pool.tile([nr, HALF, 128], f32, tag="ps")
                    for g in range(HALF // GRP):
                        bg = b0 + g * GRP
                        out_ap = ps[:, g * GRP:(g + 1) * GRP, 0:ncw]
                        nc.tensor.matmul(
                            out_ap, lpos, Y[:, bg:bg + GRP, rw:rw + ncw],
                            start=True, stop=False)
                        nc.tensor.matmul(
                            out_ap, lneg, Y[:, bg:bg + GRP, 0:ncw],
                            start=False, stop=True)
                    sc = scores_pool.tile([nr, HALF, ncw], f32, tag="sc")
                    nc.scalar.copy(sc[:], ps[:, :, 0:ncw])
                    dst = (out_ext[b0:b0 + HALF, FLAT_OFF[r]:FLAT_OFF[r + 1]]
                           .rearrange("b (i j) -> i b j", i=nr, j=ncw))
                    nc.sync.dma_start(out=dst, in_=sc[:])
    nc.compile()
    return nc


def _run_device(x, trace=False, **run_kwargs):
    """x: (256, 1, 112, 112) f32 -> all_scores (256, TOTAL) f32."""
    from concourse.bass_utils import run_bass_kernel_spmd

    key = "nc"
    if key not in _COMPILED:
        _COMPILED[key] = _build_bass()
    nc = _COMPILED[key]

    apos, aneg = _build_consts()
    xs = np.ascontiguousarray(x.reshape(BATCH, FEAT, FEAT).astype(np.float32))
    in_maps = [{"x": xs[c * B_PER:(c + 1) * B_PER],
                "apos": apos, "aneg": aneg} for c in range(N_CORES)]
    res = run_bass_kernel_spmd(nc, in_maps, core_ids=list(range(N_CORES)),
                               trace=trace, **run_kwargs)
    out = np.concatenate([res.results[c]["out"] for c in range(N_CORES)], 0)
    if trace:
        return out, res
    return out


# ---------------- host NMS (float32 semantics identical to reference) -------

def _nms_group(scores_sub, coords_sub, n_pick, thresh, K=4096):
    """Greedy NMS per image over a group, vectorized over the batch.

    Exact reproduction of the reference argmax loop (first-index tie-break,
    float32 IoU arithmetic), run on the top-K candidates per image; falls
    back to the full set for any image that exhausts its candidates.
    """
    B, N = scores_sub.shape
    K = min(K, N)
    f32 = np.float32
    thresh = f32(thresh)
    one = f32(1.0)

    if K < N:
        pidx = np.argpartition(-scores_sub, K - 1, axis=1)[:, :K]
    else:
        pidx = np.broadcast_to(np.arange(N), (B, N)).copy()
    # sort candidates by (score desc, index asc) so that np.argmax's
    # first-max tie-break matches the reference's full-array argmax
    order = np.lexsort((pidx, -np.take_along_axis(scores_sub, pidx, 1)), axis=1)
    pidx = np.take_along_axis(pidx, order, 1)

    s = np.take_along_axis(scores_sub, pidx, 1).astype(f32)       # (B,K)
    boxes = coords_sub[pidx].astype(f32)                          # (B,K,4)
    areas = (boxes[..., 2] - boxes[..., 0] + one) * (boxes[..., 3] - boxes[..., 1] + one)

    alive = np.ones((B, K), bool)
    last = np.zeros(B, np.int64)
    rows = np.arange(B)
    picks = np.zeros((B, n_pick), np.int64)
    exhausted = np.zeros(B, bool)
    NEG = f32(-np.inf)

    for step in range(n_pick):
        any_alive = alive.any(1)
        exhausted |= ~any_alive
        masked = np.where(alive, s, NEG)
        j = np.argmax(masked, 1)
        idx = np.where(any_alive, j, last)
        box = boxes[rows, idx]                                    # (B,4)
        lt = np.maximum(boxes[..., :2], box[:, None, :2])
        rb = np.minimum(boxes[..., 2:], box[:, None, 2:])
        w = rb[..., 0] - lt[..., 0] + one
        h = rb[..., 1] - lt[..., 1] + one
        inter = np.where((w < 0) | (h < 0), f32(0.0), w * h)
        iou = inter / (areas + areas[rows, idx][:, None] - inter)
        keep = alive & (iou <= thresh)
        keep[rows, idx] = False
        alive = np.where(any_alive[:, None], keep, alive)
        last = idx
        picks[:, step] = idx

    result = np.take_along_axis(pidx, picks, 1).astype(np.int32)

    if K < N and exhausted.any():
        for b in np.nonzero(exhausted)[0]:
            result[b] = _nms_group(scores_sub[b:b + 1], coords_sub,
                                   n_pick, thresh, K=N)[0]
    return result


def _host_nms(all_scores, coords):
    idx_parts = []
    for j, (g0, g1) in enumerate(GROUPS):
        s0 = FLAT_OFF[g0]
        s1 = FLAT_OFF[g1]
        picks = _nms_group(all_scores[:, s0:s1], coords[s0:s1],
                           N_LIST[j], IOU_THRESHS[j])
        idx_parts.append(picks + s0)
    return np.concatenate(idx_parts, 1).astype(np.int32)


def kernel(x, coords, proposalN):
    x = np.asarray(x)
    coords = np.asarray(coords, dtype=np.float32)
    assert int(proposalN) == PROPOSALN
    all_scores = _run_device(np.asarray(x, dtype=np.float32))
    proposalN_indices = _host_nms(all_scores, coords)
    proposalN_windows_scores = np.take_along_axis(all_scores, proposalN_indices, 1)
    return proposalN_indices, proposalN_windows_scores, all_scores


# revision 9
# speedup vs baseline: 1.7787x; 1.7787x over previous
"""Trainium2 Bass kernel for multi-scale average-pool window scoring + NMS proposals.

Problem: x (256,1,112,112) f32 -> 13-ratio sliding-window average scores
(256, 96981), then greedy NMS per 3 ratio-groups picking (2,3,2) boxes.

Device strategy (pure data parallelism, 32 images per core x 8 cores):
  - DVE tensor_tensor_scan: per-row prefix sums  Y[h, j'] = sum_{w<j'} X[h, w]
  - TensorE: scores_r = (s_r*A_rh)^T @ Y[:, j+rw]  -  (s_r*A_rh)^T @ Y[:, j]
    via two PSUM-accumulating matmuls with banded constant stationaries
    (window-sum over rows h as a matmul against the column prefix sums).
  - ScalarE: PSUM -> SBUF evacuation.
  - DMA out the (32, 96981) score shard.
NMS (256x7 int32 indices, 0.006% of output bytes) is computed on host from
the full returned scores with float32 semantics matching the reference.
"""

import os
import numpy as np

# ---------------- problem geometry (hardcoded, matches the nn.Module) -------
FEAT = 112
PSTRIDE = 4
RATIOS = [(16, 16), (12, 20), (20, 12),
          (24, 24), (20, 28), (28, 20),
          (32, 32), (24, 40), (40, 24), (28, 40), (40, 28), (28, 36), (36, 28)]
GROUPS = [(0, 3), (3, 6), (6, 13)]
N_LIST = [2, 3, 2]
IOU_THRESHS = [0.25, 0.25, 0.25]
BATCH = 256
WINDOW_NUMS = [(FEAT - rh + 1) * (FEAT - rw + 1) for rh, rw in RATIOS]
TOTAL = sum(WINDOW_NUMS)  # 96981
PROPOSALN = sum(N_LIST)   # 7

N_CORES = 8
B_PER = BATCH // N_CORES  # 32

NR = [FEAT - rh + 1 for rh, _ in RATIOS]
NC = [FEAT - rw + 1 for _, rw in RATIOS]
NR_OFF = np.cumsum([0] + NR).tolist()   # column offsets into packed A
NRSUM = NR_OFF[-1]                      # 1144
FLAT_OFF = np.cumsum([0] + WINDOW_NUMS).tolist()

# device out layout: per (ratio, half) block of [nr, 16, nc] in natural
# (i, img, j) order — contiguous DMA; host transposes back to (img, i*j)
HALF = 16
N_HALF = B_PER // HALF
BLK = [NR[r] * HALF * NC[r] for r in range(len(RATIOS))]
BLK_OFF = []
_o = 0
for r in range(len(RATIOS)):
    for h in range(N_HALF):
        BLK_OFF.append(_o)
        _o += BLK[r]
OUT2 = _o  # == B_PER * TOTAL

_COMPILED = {}


def _build_consts():
    """Packed banded stationary matrices, scaled by +/- 1/(rh*rw)."""
    apos = np.zeros((FEAT, NRSUM), np.float32)
    for r, (rh, rw) in enumerate(RATIOS):
        s = 1.0 / float(rh * rw)
        for i in range(NR[r]):
            apos[i:i + rh, NR_OFF[r] + i] = s
    return apos, -apos


def _build_bass():
    import concourse.bass as bass
    import concourse.bacc as bacc
    import concourse.tile as tile
    import concourse.mybir as mybir

    f32 = mybir.dt.float32
    nc = bacc.Bacc()
    x_ext = nc.declare_dram_parameter("x", [B_PER, FEAT, FEAT], f32, isOutput=False)
    apos_ext = nc.declare_dram_parameter("apos", [FEAT, NRSUM], f32, isOutput=False)
    aneg_ext = nc.declare_dram_parameter("aneg", [FEAT, NRSUM], f32, isOutput=False)
    out_ext = nc.declare_dram_parameter("out", [OUT2], f32, isOutput=True)

    with tile.TileContext(nc) as tc:
        with tc.tile_pool(name="persist", bufs=1) as persist, \
             tc.tile_pool(name="scores", bufs=3) as scores_pool, \
             tc.tile_pool(name="psum", bufs=2, space="PSUM") as psum_pool:

            # constants; staged through a DVE copy so matmuls' lhsT + rhs
            # dependencies collapse onto one semaphore (S3_LW allows a
            # single external sync wait on LDWEIGHTS-carrying matmuls)
            apos_in = persist.tile([FEAT, NRSUM], f32, tag="apos_in")
            aneg_in = persist.tile([FEAT, NRSUM], f32, tag="aneg_in")
            apos = persist.tile([FEAT, NRSUM], f32, tag="apos")
            aneg = persist.tile([FEAT, NRSUM], f32, tag="aneg")
            nc.sync.dma_start(out=apos_in[:], in_=apos_ext[:])
            nc.sync.dma_start(out=aneg_in[:], in_=aneg_ext[:])
            nc.vector.tensor_copy(apos[:], apos_in[:])
            nc.vector.tensor_copy(aneg[:], aneg_in[:])

            # input images: [h=112 partitions, b=32, w=112]
            X = persist.tile([FEAT, B_PER, FEAT], f32, tag="X")
            nc.sync.dma_start(out=X[:], in_=x_ext.rearrange("b h w -> h b w"))

            # row prefix sums with leading zero column: Y[h, b, j'],
            # Y[:, :, 0] = 0, Y[:, :, 1+w] = cumsum_w
            Y = persist.tile([FEAT, B_PER, FEAT + 1], f32, tag="Y")
            nc.vector.memset(Y[:, :, 0], 0.0)
            for b in range(B_PER):
                nc.vector.tensor_tensor_scan(
                    Y[:, b, 1:FEAT + 1], X[:, b, :], X[:, b, :],
                    initial=0.0,
                    op0=mybir.AluOpType.add, op1=mybir.AluOpType.bypass)

            GRP = 4     # images per matmul (N = GRP*nc <= 512)
            blk_i = 0
            for r, (rh, rw) in enumerate(RATIOS):
                nr, ncw = NR[r], NC[r]
                lpos = apos[:, NR_OFF[r]:NR_OFF[r] + nr]
                lneg = aneg[:, NR_OFF[r]:NR_OFF[r] + nr]
                for h in range(N_HALF):
                    b0 = h * HALF
                    ps = psum_pool.tile([nr, HALF, 128], f32, tag="ps")
                    for g in range(HALF // GRP):
                        bg = b0 + g * GRP
                        out_ap = ps[:, g * GRP:(g + 1) * GRP, 0:ncw]
                        nc.tensor.matmul(
                            out_ap, lpos, Y[:, bg:bg + GRP, rw:rw + ncw],
                            start=True, stop=False)
                        nc.tensor.matmul(
                            out_ap, lneg, Y[:, bg:bg + GRP, 0:ncw],
                            start=False, stop=True)
                    sc = scores_pool.tile([nr, HALF, ncw], f32, tag="sc")
                    nc.scalar.copy(sc[:], ps[:, :, 0:ncw])
                    off = BLK_OFF[blk_i]
                    blk_i += 1
                    dst = (out_ext[off:off + nr * HALF * ncw]
                           .rearrange("(i b j) -> i b j", i=nr, b=HALF, j=ncw))
                    nc.sync.dma_start(out=dst, in_=sc[:])
    nc.compile()
    return nc


def _run_device(x, trace=False, **run_kwargs):
    """x: (256, 1, 112, 112) f32 -> all_scores (256, TOTAL) f32."""
    from concourse.bass_utils import run_bass_kernel_spmd

    key = "nc"
    if key not in _COMPILED:
        _COMPILED[key] = _build_bass()
    nc = _COMPILED[key]

    apos, aneg = _build_consts()
    xs = np.ascontiguousarray(x.reshape(BATCH, FEAT, FEAT).astype(np.float32))
    in_maps = [{"x": xs[c * B_PER:(c + 1) * B_PER],
                "apos": apos, "aneg": aneg} for c in range(N_CORES)]
    res = run_bass_kernel_spmd(nc, in_maps, core_ids=list(range(N_CORES)),
                               trace=trace, **run_kwargs)
    out = np.empty((BATCH, TOTAL), np.float32)
    for c in range(N_CORES):
        raw = res.results[c]["out"]
        blk_i = 0
        for r in range(len(RATIOS)):
            nr, ncw = NR[r], NC[r]
            for h in range(N_HALF):
                off = BLK_OFF[blk_i]
                blk_i += 1
                blk = raw[off:off + nr * HALF * ncw].reshape(nr, HALF, ncw)
                out[c * B_PER + h * HALF:c * B_PER + (h + 1) * HALF,
                    FLAT_OFF[r]:FLAT_OFF[r + 1]] = \
                    blk.transpose(1, 0, 2).reshape(HALF, nr * ncw)
    if trace:
        return out, res
    return out


# ---------------- host NMS (float32 semantics identical to reference) -------

def _nms_group(scores_sub, coords_sub, n_pick, thresh, K=4096):
    """Greedy NMS per image over a group, vectorized over the batch.

    Exact reproduction of the reference argmax loop (first-index tie-break,
    float32 IoU arithmetic), run on the top-K candidates per image; falls
    back to the full set for any image that exhausts its candidates.
    """
    B, N = scores_sub.shape
    K = min(K, N)
    f32 = np.float32
    thresh = f32(thresh)
    one = f32(1.0)

    if K < N:
        pidx = np.argpartition(-scores_sub, K - 1, axis=1)[:, :K]
    else:
        pidx = np.broadcast_to(np.arange(N), (B, N)).copy()
    # sort candidates by (score desc, index asc) so that np.argmax's
    # first-max tie-break matches the reference's full-array argmax
    order = np.lexsort((pidx, -np.take_along_axis(scores_sub, pidx, 1)), axis=1)
    pidx = np.take_along_axis(pidx, order, 1)

    s = np.take_along_axis(scores_sub, pidx, 1).astype(f32)       # (B,K)
    boxes = coords_sub[pidx].astype(f32)                          # (B,K,4)
    areas = (boxes[..., 2] - boxes[..., 0] + one) * (boxes[..., 3] - boxes[..., 1] + one)

    alive = np.ones((B, K), bool)
    last = np.zeros(B, np.int64)
    rows = np.arange(B)
    picks = np.zeros((B, n_pick), np.int64)
    exhausted = np.zeros(B, bool)
    NEG = f32(-np.inf)

    for step in range(n_pick):
        any_alive = alive.any(1)
        exhausted |= ~any_alive
        masked = np.where(alive, s, NEG)
        j = np.argmax(masked, 1)
        idx = np.where(any_alive, j, last)
        box = boxes[rows, idx]                                    # (B,4)
        lt = np.maximum(boxes[..., :2], box[:, None, :2])
        rb = np.minimum(boxes[..., 2:], box[:, None, 2:])
        w = rb[..., 0] - lt[..., 0] + one
        h = rb[..., 1] - lt[..., 1] + one
        inter = np.where((w < 0) | (h < 0), f32(0.0), w * h)
        iou = inter / (areas + areas[rows, idx][:, None] - inter)
        keep = alive & (iou <= thresh)
        keep[rows, idx] = False
        alive = np.where(any_alive[:, None], keep, alive)
        last = idx
        picks[:, step] = idx

    result = np.take_along_axis(pidx, picks, 1).astype(np.int32)

    if K < N and exhausted.any():
        for b in np.nonzero(exhausted)[0]:
            result[b] = _nms_group(scores_sub[b:b + 1], coords_sub,
                                   n_pick, thresh, K=N)[0]
    return result


def _host_nms(all_scores, coords):
    idx_parts = []
    for j, (g0, g1) in enumerate(GROUPS):
        s0 = FLAT_OFF[g0]
        s1 = FLAT_OFF[g1]
        picks = _nms_group(all_scores[:, s0:s1], coords[s0:s1],
                           N_LIST[j], IOU_THRESHS[j])
        idx_parts.append(picks + s0)
    return np.concatenate(idx_parts, 1).astype(np.int32)


def kernel(x, coords, proposalN):
    x = np.asarray(x)
    coords = np.asarray(coords, dtype=np.float32)
    assert int(proposalN) == PROPOSALN
    all_scores = _run_device(np.asarray(x, dtype=np.float32))
    proposalN_indices = _host_nms(all_scores, coords)
    proposalN_windows_scores = np.take_along_axis(all_scores, proposalN_indices, 1)
    return proposalN_indices, proposalN_windows_scores, all_scores


# revision 10
# speedup vs baseline: 2.4656x; 1.3862x over previous
"""Trainium2 Bass kernel for multi-scale average-pool window scoring + NMS proposals.

Problem: x (256,1,112,112) f32 -> 13-ratio sliding-window average scores
(256, 96981), then greedy NMS per 3 ratio-groups picking (2,3,2) boxes.

Device strategy (pure data parallelism, 32 images per core x 8 cores):
  - DVE tensor_tensor_scan: per-row prefix sums  Y[h, j'] = sum_{w<j'} X[h, w]
  - TensorE: scores_r = (s_r*A_rh)^T @ Y[:, j+rw]  -  (s_r*A_rh)^T @ Y[:, j]
    via two PSUM-accumulating matmuls with banded constant stationaries
    (window-sum over rows h as a matmul against the column prefix sums).
  - ScalarE: PSUM -> SBUF evacuation.
  - DMA out the (32, 96981) score shard.
NMS (256x7 int32 indices, 0.006% of output bytes) is computed on host from
the full returned scores with float32 semantics matching the reference.
"""

import os
import numpy as np

# ---------------- problem geometry (hardcoded, matches the nn.Module) -------
FEAT = 112
PSTRIDE = 4
RATIOS = [(16, 16), (12, 20), (20, 12),
          (24, 24), (20, 28), (28, 20),
          (32, 32), (24, 40), (40, 24), (28, 40), (40, 28), (28, 36), (36, 28)]
GROUPS = [(0, 3), (3, 6), (6, 13)]
N_LIST = [2, 3, 2]
IOU_THRESHS = [0.25, 0.25, 0.25]
BATCH = 256
WINDOW_NUMS = [(FEAT - rh + 1) * (FEAT - rw + 1) for rh, rw in RATIOS]
TOTAL = sum(WINDOW_NUMS)  # 96981
PROPOSALN = sum(N_LIST)   # 7

N_CORES = 8
B_PER = BATCH // N_CORES  # 32

NR = [FEAT - rh + 1 for rh, _ in RATIOS]
NC = [FEAT - rw + 1 for _, rw in RATIOS]
NR_OFF = np.cumsum([0] + NR).tolist()   # column offsets into packed A
NRSUM = NR_OFF[-1]                      # 1144
FLAT_OFF = np.cumsum([0] + WINDOW_NUMS).tolist()

# device out layout: per (ratio, half) block of [nr, 16, nc] in natural
# (i, img, j) order — contiguous DMA; host transposes back to (img, i*j)
HALF = 16
N_HALF = B_PER // HALF
BLK = [NR[r] * HALF * NC[r] for r in range(len(RATIOS))]
BLK_OFF = []
_o = 0
for r in range(len(RATIOS)):
    for h in range(N_HALF):
        BLK_OFF.append(_o)
        _o += BLK[r]
OUT2 = _o  # == B_PER * TOTAL

_COMPILED = {}


def _build_consts():
    """Packed banded stationary matrices, scaled by +/- 1/(rh*rw)."""
    apos = np.zeros((FEAT, NRSUM), np.float32)
    for r, (rh, rw) in enumerate(RATIOS):
        s = 1.0 / float(rh * rw)
        for i in range(NR[r]):
            apos[i:i + rh, NR_OFF[r] + i] = s
    return apos, -apos


def _build_bass():
    import concourse.bass as bass
    import concourse.bacc as bacc
    import concourse.tile as tile
    import concourse.mybir as mybir

    f32 = mybir.dt.float32
    nc = bacc.Bacc()
    x_ext = nc.declare_dram_parameter("x", [B_PER, FEAT, FEAT], f32, isOutput=False)
    apos_ext = nc.declare_dram_parameter("apos", [FEAT, NRSUM], f32, isOutput=False)
    aneg_ext = nc.declare_dram_parameter("aneg", [FEAT, NRSUM], f32, isOutput=False)
    out_ext = nc.declare_dram_parameter("out", [OUT2], f32, isOutput=True)

    with tile.TileContext(nc) as tc:
        with tc.tile_pool(name="persist", bufs=1) as persist, \
             tc.tile_pool(name="scores", bufs=3) as scores_pool, \
             tc.tile_pool(name="psum", bufs=2, space="PSUM") as psum_pool:

            # constants; staged through a DVE copy so matmuls' lhsT + rhs
            # dependencies collapse onto one semaphore (S3_LW allows a
            # single external sync wait on LDWEIGHTS-carrying matmuls)
            apos_in = persist.tile([FEAT, NRSUM], f32, tag="apos_in")
            aneg_in = persist.tile([FEAT, NRSUM], f32, tag="aneg_in")
            apos = persist.tile([FEAT, NRSUM], f32, tag="apos")
            aneg = persist.tile([FEAT, NRSUM], f32, tag="aneg")
            nc.sync.dma_start(out=apos_in[:], in_=apos_ext[:])
            nc.sync.dma_start(out=aneg_in[:], in_=aneg_ext[:])
            nc.vector.tensor_copy(apos[:], apos_in[:])
            nc.vector.tensor_copy(aneg[:], aneg_in[:])

            # input images: [h=112 partitions, b=32, w=112]
            X = persist.tile([FEAT, B_PER, FEAT], f32, tag="X")
            nc.gpsimd.dma_start(out=X[:], in_=x_ext.rearrange("b h w -> h b w"))

            # row prefix sums with leading zero column: Y[h, b, j'],
            # Y[:, :, 0] = 0, Y[:, :, 1+w] = cumsum_w
            Y = persist.tile([FEAT, B_PER, FEAT + 1], f32, tag="Y")
            nc.vector.memset(Y[:, :, 0], 0.0)
            for b in range(B_PER):
                nc.vector.tensor_tensor_scan(
                    Y[:, b, 1:FEAT + 1], X[:, b, :], X[:, b, :],
                    initial=0.0,
                    op0=mybir.AluOpType.add, op1=mybir.AluOpType.bypass)

            GRP = 4     # images per matmul (N = GRP*nc <= 512)
            blk_i = 0
            for r, (rh, rw) in enumerate(RATIOS):
                nr, ncw = NR[r], NC[r]
                lpos = apos[:, NR_OFF[r]:NR_OFF[r] + nr]
                lneg = aneg[:, NR_OFF[r]:NR_OFF[r] + nr]
                for h in range(N_HALF):
                    b0 = h * HALF
                    ps = psum_pool.tile([nr, HALF, 128], f32, tag="ps")
                    for g in range(HALF // GRP):
                        bg = b0 + g * GRP
                        out_ap = ps[:, g * GRP:(g + 1) * GRP, 0:ncw]
                        nc.tensor.matmul(
                            out_ap, lpos, Y[:, bg:bg + GRP, rw:rw + ncw],
                            start=True, stop=False)
                        nc.tensor.matmul(
                            out_ap, lneg, Y[:, bg:bg + GRP, 0:ncw],
                            start=False, stop=True)
                    sc = scores_pool.tile([nr, HALF, ncw], f32, tag="sc")
                    nc.scalar.copy(sc[:], ps[:, :, 0:ncw])
                    off = BLK_OFF[blk_i]
                    blk_i += 1
                    dst = (out_ext[off:off + nr * HALF * ncw]
                           .rearrange("(i b j) -> i b j", i=nr, b=HALF, j=ncw))
                    nc.gpsimd.dma_start(out=dst, in_=sc[:])
    nc.compile()
    return nc


def _run_device(x, trace=False, **run_kwargs):
    """x: (256, 1, 112, 112) f32 -> all_scores (256, TOTAL) f32."""
    from concourse.bass_utils import run_bass_kernel_spmd

    key = "nc"
    if key not in _COMPILED:
        _COMPILED[key] = _build_bass()
    nc = _COMPILED[key]

    apos, aneg = _build_consts()
    xs = np.ascontiguousarray(x.reshape(BATCH, FEAT, FEAT).astype(np.float32))
    in_maps = [{"x": xs[c * B_PER:(c + 1) * B_PER],
                "apos": apos, "aneg": aneg} for c in range(N_CORES)]
    res = run_bass_kernel_spmd(nc, in_maps, core_ids=list(range(N_CORES)),
                               trace=trace, **run_kwargs)
    out = np.empty((BATCH, TOTAL), np.float32)
    for c in range(N_CORES):
        raw = res.results[c]["out"]
        blk_i = 0
        for r in range(len(RATIOS)):
            nr, ncw = NR[r], NC[r]
            for h in range(N_HALF):
                off = BLK_OFF[blk_i]
                blk_i += 1
                blk = raw[off:off + nr * HALF * ncw].reshape(nr, HALF, ncw)
                out[c * B_PER + h * HALF:c * B_PER + (h + 1) * HALF,
                    FLAT_OFF[r]:FLAT_OFF[r + 1]] = \
                    blk.transpose(1, 0, 2).reshape(HALF, nr * ncw)
    if trace:
        return out, res
    return out


# ---------------- host NMS (float32 semantics identical to reference) -------

def _nms_group(scores_sub, coords_sub, n_pick, thresh, K=4096):
    """Greedy NMS per image over a group, vectorized over the batch.

    Exact reproduction of the reference argmax loop (first-index tie-break,
    float32 IoU arithmetic), run on the top-K candidates per image; falls
    back to the full set for any image that exhausts its candidates.
    """
    B, N = scores_sub.shape
    K = min(K, N)
    f32 = np.float32
    thresh = f32(thresh)
    one = f32(1.0)

    if K < N:
        pidx = np.argpartition(-scores_sub, K - 1, axis=1)[:, :K]
    else:
        pidx = np.broadcast_to(np.arange(N), (B, N)).copy()
    # sort candidates by (score desc, index asc) so that np.argmax's
    # first-max tie-break matches the reference's full-array argmax
    order = np.lexsort((pidx, -np.take_along_axis(scores_sub, pidx, 1)), axis=1)
    pidx = np.take_along_axis(pidx, order, 1)

    s = np.take_along_axis(scores_sub, pidx, 1).astype(f32)       # (B,K)
    boxes = coords_sub[pidx].astype(f32)                          # (B,K,4)
    areas = (boxes[..., 2] - boxes[..., 0] + one) * (boxes[..., 3] - boxes[..., 1] + one)

    alive = np.ones((B, K), bool)
    last = np.zeros(B, np.int64)
    rows = np.arange(B)
    picks = np.zeros((B, n_pick), np.int64)
    exhausted = np.zeros(B, bool)
    NEG = f32(-np.inf)

    for step in range(n_pick):
        any_alive = alive.any(1)
        exhausted |= ~any_alive
        masked = np.where(alive, s, NEG)
        j = np.argmax(masked, 1)
        idx = np.where(any_alive, j, last)
        box = boxes[rows, idx]                                    # (B,4)
        lt = np.maximum(boxes[..., :2], box[:, None, :2])
        rb = np.minimum(boxes[..., 2:], box[:, None, 2:])
        w = rb[..., 0] - lt[..., 0] + one
        h = rb[..., 1] - lt[..., 1] + one
        inter = np.where((w < 0) | (h < 0), f32(0.0), w * h)
        iou = inter / (areas + areas[rows, idx][:, None] - inter)
        keep = alive & (iou <= thresh)
        keep[rows, idx] = False
        alive = np.where(any_alive[:, None], keep, alive)
        last = idx
        picks[:, step] = idx

    result = np.take_along_axis(pidx, picks, 1).astype(np.int32)

    if K < N and exhausted.any():
        for b in np.nonzero(exhausted)[0]:
            result[b] = _nms_group(scores_sub[b:b + 1], coords_sub,
                                   n_pick, thresh, K=N)[0]
    return result


def _host_nms(all_scores, coords):
    idx_parts = []
    for j, (g0, g1) in enumerate(GROUPS):
        s0 = FLAT_OFF[g0]
        s1 = FLAT_OFF[g1]
        picks = _nms_group(all_scores[:, s0:s1], coords[s0:s1],
                           N_LIST[j], IOU_THRESHS[j])
        idx_parts.append(picks + s0)
    return np.concatenate(idx_parts, 1).astype(np.int32)


def kernel(x, coords, proposalN):
    x = np.asarray(x)
    coords = np.asarray(coords, dtype=np.float32)
    assert int(proposalN) == PROPOSALN
    all_scores = _run_device(np.asarray(x, dtype=np.float32))
    proposalN_indices = _host_nms(all_scores, coords)
    proposalN_windows_scores = np.take_along_axis(all_scores, proposalN_indices, 1)
    return proposalN_indices, proposalN_windows_scores, all_scores


# revision 34
# speedup vs baseline: 3.6265x; 1.4709x over previous
"""Trainium2 Bass kernel for multi-scale average-pool window scoring + NMS proposals.

Problem: x (256,1,112,112) f32 -> 13-ratio sliding-window average scores
(256, 96981), then greedy NMS per 3 ratio-groups picking (2,3,2) boxes.

Device strategy (pure data parallelism, 32 images per core x 8 cores),
pipelined per 8-image quarter / 16-image half:
  - DVE tensor_tensor_scan: per-row prefix sums  Y[h, b, j'] = sum_{w<j'} X
  - hi/lo bf16 split (Y == Yh + Yl to ~18 mantissa bits; ScalarE + DVE)
  - TensorE: E_rh = A_rh^T @ Yh + A_rh^T @ Yl, accumulated in fp32 PSUM,
    one pair of 1-cycle/col bf16 matmuls per unique window height rh
    (binary banded A is exact in bf16; N = 4 images x 114 cols = 456)
  - ScalarE evacuates the unshifted E columns PSUM->SBUF; DVE then forms
    the unscaled scores E[i, j+rw] - E[i, j] reading the shifted side
    straight from PSUM (keeps the gpsimd-shared SBUF port free for
    SWDGE DMA descriptor generation)
  - gpsimd SWDGE DMAs stream each (ratio, 16-image) block to HBM in the
    device-natural [i, img, j] order (contiguous ~5.5KB runs/partition,
    fanned across all 16 SDMA engines)
Host: unshard + transpose blocks back to (img, i*j) while applying the
1/(rh*rw) average constant. NMS (256x7 int32 indices, 0.006% of output
bytes) runs on host from the full returned scores with float32 semantics
identical to the reference (verified bit-exact picks on multiple seeds).
"""

import numpy as np

# ---------------- problem geometry (hardcoded, matches the nn.Module) -------
FEAT = 112
PSTRIDE = 4
RATIOS = [(16, 16), (12, 20), (20, 12),
          (24, 24), (20, 28), (28, 20),
          (32, 32), (24, 40), (40, 24), (28, 40), (40, 28), (28, 36), (36, 28)]
GROUPS = [(0, 3), (3, 6), (6, 13)]
N_LIST = [2, 3, 2]
IOU_THRESHS = [0.25, 0.25, 0.25]
BATCH = 256
WINDOW_NUMS = [(FEAT - rh + 1) * (FEAT - rw + 1) for rh, rw in RATIOS]
TOTAL = sum(WINDOW_NUMS)  # 96981
PROPOSALN = sum(N_LIST)   # 7

N_CORES = 8
B_PER = BATCH // N_CORES  # 32

NR = [FEAT - rh + 1 for rh, _ in RATIOS]
NC = [FEAT - rw + 1 for _, rw in RATIOS]
FLAT_OFF = np.cumsum([0] + WINDOW_NUMS).tolist()

# unique window heights; stationary A matrices are shared per rh
RHU = sorted({rh for rh, _ in RATIOS})          # [12,16,20,24,28,32,36,40]
NRU = [FEAT - rh + 1 for rh in RHU]
RHU_OFF = np.cumsum([0] + NRU).tolist()         # col offsets into packed A
NRUSUM = RHU_OFF[-1]                            # 696
RH_IDX = {rh: u for u, rh in enumerate(RHU)}

# device out layout: per (ratio, half) block of [nr, 16, nc] in natural
# (i, img, j) order — contiguous DMA; host transposes back to (img, i*j)
HALF = 16
N_HALF = B_PER // HALF
BLK = [NR[r] * HALF * NC[r] for r in range(len(RATIOS))]
BLK_OFF = []
_o = 0
for r in range(len(RATIOS)):
    for h in range(N_HALF):
        BLK_OFF.append(_o)
        _o += BLK[r]
OUT2 = _o  # == B_PER * TOTAL

_COMPILED = {}


def _build_consts():
    """Packed banded binary stationaries, one block per unique rh."""
    ab = np.zeros((FEAT, NRUSUM), np.float32)
    for u, rh in enumerate(RHU):
        for i in range(NRU[u]):
            ab[i:i + rh, RHU_OFF[u] + i] = 1.0
    return ab


def _build_bass():
    import concourse.bass as bass
    import concourse.bacc as bacc
    import concourse.tile as tile
    import concourse.mybir as mybir

    f32 = mybir.dt.float32
    bf16 = mybir.dt.bfloat16
    f32r = mybir.dt.float32r
    nc = bacc.Bacc()
    x_ext = nc.declare_dram_parameter("x", [B_PER, FEAT, FEAT], f32, isOutput=False)
    ab_ext = nc.declare_dram_parameter("ab", [FEAT, NRUSUM], f32, isOutput=False)
    out_ext = nc.declare_dram_parameter("out", [OUT2], f32, isOutput=True)

    WPAD = FEAT + 2  # 114 prefix-sum cols (col 0 == 0, col 113 pad);
    # fp32r matmul requires an even innermost free count

    with tile.TileContext(nc) as tc:
        with tc.tile_pool(name="persist", bufs=1) as persist, \
             tc.tile_pool(name="esb", bufs=6) as e_pool, \
             tc.tile_pool(name="scores", bufs=12) as scores_pool, \
             tc.tile_pool(name="psum", bufs=2, space="PSUM") as psum_pool:

            # constants; staged through a DVE copy so matmuls' lhsT + rhs
            # dependencies collapse onto one semaphore (S3_LW allows a
            # single external sync wait on LDWEIGHTS-carrying matmuls)
            ab_in = persist.tile([FEAT, NRUSUM], f32, tag="ab_in")
            ab = persist.tile([FEAT, NRUSUM], bf16, tag="ab")
            nc.sync.dma_start(out=ab_in[:], in_=ab_ext[:])

            # quarter-granularity input pipeline (8 images per tile):
            # X DMA -> scans -> hi/lo bf16 split; lets the first matmuls
            # start after 8 images and removes the half-boundary stall
            QN = 4
            QI = B_PER // QN  # 8
            X = [persist.tile([FEAT, QI, FEAT], f32, name=f"X{q}", tag=f"X{q}")
                 for q in range(QN)]
            Y = [persist.tile([FEAT, QI, WPAD], f32, name=f"Y{q}", tag=f"Y{q}")
                 for q in range(QN)]
            Yh = [persist.tile([FEAT, QI, WPAD], bf16, name=f"Yh{q}", tag=f"Yh{q}")
                  for q in range(QN)]
            Yl = [persist.tile([FEAT, QI, WPAD], bf16, name=f"Yl{q}", tag=f"Yl{q}")
                  for q in range(QN)]
            for q in range(QN):
                b0 = q * QI
                nc.gpsimd.dma_start(
                    out=X[q][:],
                    in_=x_ext[b0:b0 + QI].rearrange("b h w -> h b w"))
                nc.vector.memset(Y[q][:, :, 0], 0.0)
                nc.vector.memset(Y[q][:, :, WPAD - 1], 0.0)
                for b in range(QI):
                    nc.vector.tensor_tensor_scan(
                        Y[q][:, b, 1:FEAT + 1], X[q][:, b, :], X[q][:, b, :],
                        initial=0.0,
                        op0=mybir.AluOpType.add, op1=mybir.AluOpType.bypass)
                nc.scalar.copy(Yh[q][:], Y[q][:])
                nc.vector.tensor_sub(Yl[q][:], Y[q][:], Yh[q][:])
                if q == 0:
                    nc.vector.tensor_copy(ab[:], ab_in[:])

            GRP = 4     # images per matmul (N = GRP*114 = 456 <= 512 fp32 psum)
            ratios_of = {u: [r for r, (rh, _) in enumerate(RATIOS)
                             if rh == RHU[u]] for u in range(len(RHU))}
            blk_map = {(r, h): BLK_OFF[r * N_HALF + h]
                       for r in range(len(RATIOS)) for h in range(N_HALF)}

            for h in range(N_HALF):
                for u, rh in enumerate(RHU):
                    nr = NRU[u]
                    lhs = ab[:, RHU_OFF[u]:RHU_OFF[u] + nr]
                    max_nc = max(NC[r] for r in ratios_of[u])
                    # E_rh = A_rh^T @ (Yh + Yl) for 16 images: [nr, 16, 128pad]
                    ps = psum_pool.tile([128, HALF, 128], f32, tag="ps")
                    for g in range(HALF // GRP):
                        q = 2 * h + g // 2
                        bg = (g % 2) * GRP
                        out_ap = ps[0:nr, g * GRP:(g + 1) * GRP, 0:WPAD]
                        nc.tensor.matmul(out_ap, lhs,
                                         Yh[q][:, bg:bg + GRP, :],
                                         start=True, stop=False)
                        nc.tensor.matmul(out_ap, lhs,
                                         Yl[q][:, bg:bg + GRP, :],
                                         start=False, stop=True)
                    # evac only the in1 columns; the shifted in0 side is read
                    # straight from PSUM (keeps the shared SBUF port free for
                    # SWDGE descriptor generation)
                    esb = e_pool.tile([128, HALF, FEAT + 1], f32, tag="esb")
                    nc.scalar.copy(esb[0:nr, :, 0:max_nc], ps[0:nr, :, 0:max_nc])
                    for r in ratios_of[u]:
                        rw = RATIOS[r][1]
                        ncw = NC[r]
                        sc = scores_pool.tile([128, HALF, ncw], f32, tag="sc")
                        nc.vector.tensor_sub(
                            sc[0:nr, :, :], ps[0:nr, :, rw:rw + ncw],
                            esb[0:nr, :, 0:ncw])
                        off = blk_map[(r, h)]
                        dst = (out_ext[off:off + nr * HALF * ncw]
                               .rearrange("(i b j) -> i b j",
                                          i=nr, b=HALF, j=ncw))
                        nc.gpsimd.dma_start(out=dst, in_=sc[0:nr, :, :])
    nc.compile()
    return nc


def _run_device(x, trace=False, **run_kwargs):
    """x: (256, 1, 112, 112) f32 -> all_scores (256, TOTAL) f32."""
    from concourse.bass_utils import run_bass_kernel_spmd

    key = "nc"
    if key not in _COMPILED:
        _COMPILED[key] = _build_bass()
    nc = _COMPILED[key]

    ab = _build_consts()
    xs = np.ascontiguousarray(x.reshape(BATCH, FEAT, FEAT).astype(np.float32))
    in_maps = [{"x": xs[c * B_PER:(c + 1) * B_PER], "ab": ab}
               for c in range(N_CORES)]
    res = run_bass_kernel_spmd(nc, in_maps, core_ids=list(range(N_CORES)),
                               trace=trace, **run_kwargs)
    out = np.empty((BATCH, TOTAL), np.float32)
    for c in range(N_CORES):
        raw = res.results[c]["out"]
        blk_i = 0
        for r, (rh, rw) in enumerate(RATIOS):
            nr, ncw = NR[r], NC[r]
            s = np.float32(1.0 / (rh * rw))
            for h in range(N_HALF):
                off = BLK_OFF[blk_i]
                blk_i += 1
                blk = raw[off:off + nr * HALF * ncw].reshape(nr, HALF, ncw)
                out[c * B_PER + h * HALF:c * B_PER + (h + 1) * HALF,
                    FLAT_OFF[r]:FLAT_OFF[r + 1]] = \
                    (blk.transpose(1, 0, 2).reshape(HALF, nr * ncw) * s)
    if trace:
        return out, res
    return out


# ---------------- host NMS (float32 semantics identical to reference) -------

def _nms_group(scores_sub, coords_sub, n_pick, thresh, K=4096):
    """Greedy NMS per image over a group, vectorized over the batch.

    Exact reproduction of the reference argmax loop (first-index tie-break,
    float32 IoU arithmetic), run on the top-K candidates per image; falls
    back to the full set for any image that exhausts its candidates.
    """
    B, N = scores_sub.shape
    K = min(K, N)
    f32 = np.float32
    thresh = f32(thresh)
    one = f32(1.0)

    if K < N:
        pidx = np.argpartition(-scores_sub, K - 1, axis=1)[:, :K]
    else:
        pidx = np.broadcast_to(np.arange(N), (B, N)).copy()
    # sort candidates by (score desc, index asc) so that np.argmax's
    # first-max tie-break matches the reference's full-array argmax
    order = np.lexsort((pidx, -np.take_along_axis(scores_sub, pidx, 1)), axis=1)
    pidx = np.take_along_axis(pidx, order, 1)

    s = np.take_along_axis(scores_sub, pidx, 1).astype(f32)       # (B,K)
    boxes = coords_sub[pidx].astype(f32)                          # (B,K,4)
    areas = (boxes[..., 2] - boxes[..., 0] + one) * (boxes[..., 3] - boxes[..., 1] + one)

    alive = np.ones((B, K), bool)
    last = np.zeros(B, np.int64)
    rows = np.arange(B)
    picks = np.zeros((B, n_pick), np.int64)
    exhausted = np.zeros(B, bool)
    NEG = f32(-np.inf)

    for step in range(n_pick):
        any_alive = alive.any(1)
        exhausted |= ~any_alive
        masked = np.where(alive, s, NEG)
        j = np.argmax(masked, 1)
        idx = np.where(any_alive, j, last)
        box = boxes[rows, idx]                                    # (B,4)
        lt = np.maximum(boxes[..., :2], box[:, None, :2])
        rb = np.minimum(boxes[..., 2:], box[:, None, 2:])
        w = rb[..., 0] - lt[..., 0] + one
        h = rb[..., 1] - lt[..., 1] + one
        inter = np.where((w < 0) | (h < 0), f32(0.0), w * h)
        iou = inter / (areas + areas[rows, idx][:, None] - inter)
        keep = alive & (iou <= thresh)
        keep[rows, idx] = False
        alive = np.where(any_alive[:, None], keep, alive)
        last = idx
        picks[:, step] = idx

    result = np.take_along_axis(pidx, picks, 1).astype(np.int32)

    if K < N and exhausted.any():
        for b in np.nonzero(exhausted)[0]:
            result[b] = _nms_group(scores_sub[b:b + 1], coords_sub,
                                   n_pick, thresh, K=N)[0]
    return result


def _host_nms(all_scores, coords):
    idx_parts = []
    for j, (g0, g1) in enumerate(GROUPS):
        s0 = FLAT_OFF[g0]
        s1 = FLAT_OFF[g1]
        picks = _nms_group(all_scores[:, s0:s1], coords[s0:s1],
                           N_LIST[j], IOU_THRESHS[j])
        idx_parts.append(picks + s0)
    return np.concatenate(idx_parts, 1).astype(np.int32)


def kernel(x, coords, proposalN):
    x = np.asarray(x)
    coords = np.asarray(coords, dtype=np.float32)
    assert int(proposalN) == PROPOSALN
    all_scores = _run_device(np.asarray(x, dtype=np.float32))
    proposalN_indices = _host_nms(all_scores, coords)
    proposalN_windows_scores = np.take_along_axis(all_scores, proposalN_indices, 1)
    return proposalN_indices, proposalN_windows_scores, all_scores


# revision 35
# speedup vs baseline: 4.5120x; 1.2442x over previous
"""Trainium2 Bass kernel for multi-scale average-pool window scoring + NMS proposals.

Problem: x (256,1,112,112) f32 -> 13-ratio sliding-window average scores
(256, 96981), then greedy NMS per 3 ratio-groups picking (2,3,2) boxes.

Device strategy (pure data parallelism, 32 images per core x 8 cores),
pipelined per 8-image quarter / 16-image half:
  - DVE tensor_tensor_scan: per-row prefix sums  Y[h, b, j'] = sum_{w<j'} X
  - hi/lo bf16 split (Y == Yh + Yl to ~18 mantissa bits; ScalarE + DVE)
  - TensorE: E_rh = A_rh^T @ Yh + A_rh^T @ Yl, accumulated in fp32 PSUM,
    one pair of 1-cycle/col bf16 matmuls per unique window height rh
    (binary banded A is exact in bf16; N = 4 images x 114 cols = 456)
  - ScalarE evacuates the unshifted E columns PSUM->SBUF; DVE then forms
    the unscaled scores E[i, j+rw] - E[i, j] reading the shifted side
    straight from PSUM (keeps the gpsimd-shared SBUF port free for
    SWDGE DMA descriptor generation)
  - gpsimd SWDGE DMAs stream each (ratio, 16-image) block to HBM in the
    device-natural [i, img, j] order (contiguous ~5.5KB runs/partition,
    fanned across all 16 SDMA engines)
Host: unshard + transpose blocks back to (img, i*j) while applying the
1/(rh*rw) average constant. NMS (256x7 int32 indices, 0.006% of output
bytes) runs on host from the full returned scores with float32 semantics
identical to the reference (verified bit-exact picks on multiple seeds).
"""

import numpy as np

# ---------------- problem geometry (hardcoded, matches the nn.Module) -------
FEAT = 112
PSTRIDE = 4
RATIOS = [(16, 16), (12, 20), (20, 12),
          (24, 24), (20, 28), (28, 20),
          (32, 32), (24, 40), (40, 24), (28, 40), (40, 28), (28, 36), (36, 28)]
GROUPS = [(0, 3), (3, 6), (6, 13)]
N_LIST = [2, 3, 2]
IOU_THRESHS = [0.25, 0.25, 0.25]
BATCH = 256
WINDOW_NUMS = [(FEAT - rh + 1) * (FEAT - rw + 1) for rh, rw in RATIOS]
TOTAL = sum(WINDOW_NUMS)  # 96981
PROPOSALN = sum(N_LIST)   # 7

N_CORES = 8
B_PER = BATCH // N_CORES  # 32

NR = [FEAT - rh + 1 for rh, _ in RATIOS]
NC = [FEAT - rw + 1 for _, rw in RATIOS]
FLAT_OFF = np.cumsum([0] + WINDOW_NUMS).tolist()

# unique window heights; stationary A matrices are shared per rh
RHU = sorted({rh for rh, _ in RATIOS})          # [12,16,20,24,28,32,36,40]
NRU = [FEAT - rh + 1 for rh in RHU]
RHU_OFF = np.cumsum([0] + NRU).tolist()         # col offsets into packed A
NRUSUM = RHU_OFF[-1]                            # 696
RH_IDX = {rh: u for u, rh in enumerate(RHU)}

# device out layout: per (ratio, half) block of [nr, 16, nc] in natural
# (i, img, j) order — contiguous DMA; host transposes back to (img, i*j)
HALF = 16
N_HALF = B_PER // HALF
BLK = [NR[r] * HALF * NC[r] for r in range(len(RATIOS))]
BLK_OFF = []
_o = 0
for r in range(len(RATIOS)):
    for h in range(N_HALF):
        BLK_OFF.append(_o)
        _o += BLK[r]
OUT2 = _o  # == B_PER * TOTAL

_COMPILED = {}


def _build_consts():
    """Packed banded binary stationaries, one block per unique rh."""
    ab = np.zeros((FEAT, NRUSUM), np.float32)
    for u, rh in enumerate(RHU):
        for i in range(NRU[u]):
            ab[i:i + rh, RHU_OFF[u] + i] = 1.0
    return ab


def _build_bass():
    import concourse.bass as bass
    import concourse.bacc as bacc
    import concourse.tile as tile
    import concourse.mybir as mybir

    f32 = mybir.dt.float32
    bf16 = mybir.dt.bfloat16
    f32r = mybir.dt.float32r
    nc = bacc.Bacc()
    x_ext = nc.declare_dram_parameter("x", [B_PER, FEAT, FEAT], f32, isOutput=False)
    ab_ext = nc.declare_dram_parameter("ab", [FEAT, NRUSUM], f32, isOutput=False)
    out_ext = nc.declare_dram_parameter("out", [OUT2], f32, isOutput=True)

    WPAD = FEAT + 2  # 114 prefix-sum cols (col 0 == 0, col 113 pad);
    # fp32r matmul requires an even innermost free count

    with tile.TileContext(nc) as tc:
        with tc.tile_pool(name="persist", bufs=1) as persist, \
             tc.tile_pool(name="esb", bufs=6) as e_pool, \
             tc.tile_pool(name="scores", bufs=12) as scores_pool, \
             tc.tile_pool(name="psum", bufs=2, space="PSUM") as psum_pool:

            # constants; staged through a DVE copy so matmuls' lhsT + rhs
            # dependencies collapse onto one semaphore (S3_LW allows a
            # single external sync wait on LDWEIGHTS-carrying matmuls)
            ab_in = persist.tile([FEAT, NRUSUM], f32, tag="ab_in")
            ab = persist.tile([FEAT, NRUSUM], bf16, tag="ab")
            nc.sync.dma_start(out=ab_in[:], in_=ab_ext[:])

            # quarter-granularity input pipeline (8 images per tile):
            # X DMA -> scans -> hi/lo bf16 split; lets the first matmuls
            # start after 8 images and removes the half-boundary stall
            QN = 4
            QI = B_PER // QN  # 8
            X = [persist.tile([FEAT, QI, FEAT], f32, name=f"X{q}", tag=f"X{q}")
                 for q in range(QN)]
            Y = [persist.tile([FEAT, QI, WPAD], f32, name=f"Y{q}", tag=f"Y{q}")
                 for q in range(QN)]
            Yh = [persist.tile([FEAT, QI, WPAD], bf16, name=f"Yh{q}", tag=f"Yh{q}")
                  for q in range(QN)]
            Yl = [persist.tile([FEAT, QI, WPAD], bf16, name=f"Yl{q}", tag=f"Yl{q}")
                  for q in range(QN)]
            for q in range(QN):
                b0 = q * QI
                nc.gpsimd.dma_start(
                    out=X[q][:],
                    in_=x_ext[b0:b0 + QI].rearrange("b h w -> h b w"))
                nc.vector.memset(Y[q][:, :, 0], 0.0)
                nc.vector.memset(Y[q][:, :, WPAD - 1], 0.0)
                for b in range(QI):
                    nc.vector.tensor_tensor_scan(
                        Y[q][:, b, 1:FEAT + 1], X[q][:, b, :], X[q][:, b, :],
                        initial=0.0,
                        op0=mybir.AluOpType.add, op1=mybir.AluOpType.bypass)
                nc.scalar.copy(Yh[q][:], Y[q][:])
                nc.vector.tensor_sub(Yl[q][:], Y[q][:], Yh[q][:])
                if q == 0:
                    nc.vector.tensor_copy(ab[:], ab_in[:])

            GRP = 4     # images per matmul (N = GRP*114 = 456 <= 512 fp32 psum)
            ratios_of = {u: [r for r, (rh, _) in enumerate(RATIOS)
                             if rh == RHU[u]] for u in range(len(RHU))}
            blk_map = {(r, h): BLK_OFF[r * N_HALF + h]
                       for r in range(len(RATIOS)) for h in range(N_HALF)}

            u_order = [4, 2, 3, 7, 0, 1, 5, 6]  # big blocks first, small last
            for u in u_order:
                for h in range(N_HALF):
                    rh = RHU[u]
                    nr = NRU[u]
                    lhs = ab[:, RHU_OFF[u]:RHU_OFF[u] + nr]
                    max_nc = max(NC[r] for r in ratios_of[u])
                    # E_rh = A_rh^T @ (Yh + Yl) for 16 images: [nr, 16, 128pad]
                    ps = psum_pool.tile([128, HALF, 128], f32, tag="ps")
                    for g in range(HALF // GRP):
                        q = 2 * h + g // 2
                        bg = (g % 2) * GRP
                        out_ap = ps[0:nr, g * GRP:(g + 1) * GRP, 0:WPAD]
                        nc.tensor.matmul(out_ap, lhs,
                                         Yh[q][:, bg:bg + GRP, :],
                                         start=True, stop=False)
                        nc.tensor.matmul(out_ap, lhs,
                                         Yl[q][:, bg:bg + GRP, :],
                                         start=False, stop=True)
                    # evac only the in1 columns; the shifted in0 side is read
                    # straight from PSUM (keeps the shared SBUF port free for
                    # SWDGE descriptor generation)
                    esb = e_pool.tile([128, HALF, FEAT + 1], f32, tag="esb")
                    nc.scalar.copy(esb[0:nr, :, 0:max_nc], ps[0:nr, :, 0:max_nc])
                    for r in ratios_of[u]:
                        rw = RATIOS[r][1]
                        ncw = NC[r]
                        sc = scores_pool.tile([128, HALF, ncw], f32, tag="sc")
                        nc.vector.tensor_sub(
                            sc[0:nr, :, :], ps[0:nr, :, rw:rw + ncw],
                            esb[0:nr, :, 0:ncw])
                        off = blk_map[(r, h)]
                        dst = (out_ext[off:off + nr * HALF * ncw]
                               .rearrange("(i b j) -> i b j",
                                          i=nr, b=HALF, j=ncw))
                        nc.gpsimd.dma_start(out=dst, in_=sc[0:nr, :, :])
    nc.compile()
    return nc


def _run_device(x, trace=False, **run_kwargs):
    """x: (256, 1, 112, 112) f32 -> all_scores (256, TOTAL) f32."""
    from concourse.bass_utils import run_bass_kernel_spmd

    key = "nc"
    if key not in _COMPILED:
        _COMPILED[key] = _build_bass()
    nc = _COMPILED[key]

    ab = _build_consts()
    xs = np.ascontiguousarray(x.reshape(BATCH, FEAT, FEAT).astype(np.float32))
    in_maps = [{"x": xs[c * B_PER:(c + 1) * B_PER], "ab": ab}
               for c in range(N_CORES)]
    res = run_bass_kernel_spmd(nc, in_maps, core_ids=list(range(N_CORES)),
                               trace=trace, **run_kwargs)
    out = np.empty((BATCH, TOTAL), np.float32)
    for c in range(N_CORES):
        raw = res.results[c]["out"]
        blk_i = 0
        for r, (rh, rw) in enumerate(RATIOS):
            nr, ncw = NR[r], NC[r]
            s = np.float32(1.0 / (rh * rw))
            for h in range(N_HALF):
                off = BLK_OFF[blk_i]
                blk_i += 1
                blk = raw[off:off + nr * HALF * ncw].reshape(nr, HALF, ncw)
                out[c * B_PER + h * HALF:c * B_PER + (h + 1) * HALF,
                    FLAT_OFF[r]:FLAT_OFF[r + 1]] = \
                    (blk.transpose(1, 0, 2).reshape(HALF, nr * ncw) * s)
    if trace:
        return out, res
    return out


# ---------------- host NMS (float32 semantics identical to reference) -------

def _nms_group(scores_sub, coords_sub, n_pick, thresh, K=4096):
    """Greedy NMS per image over a group, vectorized over the batch.

    Exact reproduction of the reference argmax loop (first-index tie-break,
    float32 IoU arithmetic), run on the top-K candidates per image; falls
    back to the full set for any image that exhausts its candidates.
    """
    B, N = scores_sub.shape
    K = min(K, N)
    f32 = np.float32
    thresh = f32(thresh)
    one = f32(1.0)

    if K < N:
        pidx = np.argpartition(-scores_sub, K - 1, axis=1)[:, :K]
    else:
        pidx = np.broadcast_to(np.arange(N), (B, N)).copy()
    # sort candidates by (score desc, index asc) so that np.argmax's
    # first-max tie-break matches the reference's full-array argmax
    order = np.lexsort((pidx, -np.take_along_axis(scores_sub, pidx, 1)), axis=1)
    pidx = np.take_along_axis(pidx, order, 1)

    s = np.take_along_axis(scores_sub, pidx, 1).astype(f32)       # (B,K)
    boxes = coords_sub[pidx].astype(f32)                          # (B,K,4)
    areas = (boxes[..., 2] - boxes[..., 0] + one) * (boxes[..., 3] - boxes[..., 1] + one)

    alive = np.ones((B, K), bool)
    last = np.zeros(B, np.int64)
    rows = np.arange(B)
    picks = np.zeros((B, n_pick), np.int64)
    exhausted = np.zeros(B, bool)
    NEG = f32(-np.inf)

    for step in range(n_pick):
        any_alive = alive.any(1)
        exhausted |= ~any_alive
        masked = np.where(alive, s, NEG)
        j = np.argmax(masked, 1)
        idx = np.where(any_alive, j, last)
        box = boxes[rows, idx]                                    # (B,4)
        lt = np.maximum(boxes[..., :2], box[:, None, :2])
        rb = np.minimum(boxes[..., 2:], box[:, None, 2:])
        w = rb[..., 0] - lt[..., 0] + one
        h = rb[..., 1] - lt[..., 1] + one
        inter = np.where((w < 0) | (h < 0), f32(0.0), w * h)
        iou = inter / (areas + areas[rows, idx][:, None] - inter)
        keep = alive & (iou <= thresh)
        keep[rows, idx] = False
        alive = np.where(any_alive[:, None], keep, alive)
        last = idx
        picks[:, step] = idx

    result = np.take_along_axis(pidx, picks, 1).astype(np.int32)

    if K < N and exhausted.any():
        for b in np.nonzero(exhausted)[0]:
            result[b] = _nms_group(scores_sub[b:b + 1], coords_sub,
                                   n_pick, thresh, K=N)[0]
    return result


def _host_nms(all_scores, coords):
    idx_parts = []
    for j, (g0, g1) in enumerate(GROUPS):
        s0 = FLAT_OFF[g0]
        s1 = FLAT_OFF[g1]
        picks = _nms_group(all_scores[:, s0:s1], coords[s0:s1],
                           N_LIST[j], IOU_THRESHS[j])
        idx_parts.append(picks + s0)
    return np.concatenate(idx_parts, 1).astype(np.int32)


def kernel(x, coords, proposalN):
    x = np.asarray(x)
    coords = np.asarray(coords, dtype=np.float32)
    assert int(proposalN) == PROPOSALN
    all_scores = _run_device(np.asarray(x, dtype=np.float32))
    proposalN_indices = _host_nms(all_scores, coords)
    proposalN_windows_scores = np.take_along_axis(all_scores, proposalN_indices, 1)
    return proposalN_indices, proposalN_windows_scores, all_scores
